# revision 1
# baseline (speedup 1.0000x reference)
"""Trainium2 Bass kernel for nn_CNNVectorForm (LeNet-style CNN, batch 8192).

Pipeline per core (data-parallel over batch, 1024 images/core):
  conv 5x5 VALID (1->20ch, 28->24)  -> 2x2 maxpool -> fc1(2880->500) + relu
  -> fc2(500->10) + softmax

Device formulation:
  * All activations feature-major [features, batch] so the PE contracts
    along partitions; batch rides the free dim (512 per tile).
  * Conv as a Toeplitz matmul: for each output row r and 12-wide column
    block, one K=80 (5 rows x 16 cols of input) x M=120 matmul produces
    [20ch x 12cols, batch].  Output columns are split into even/odd
    M-tiles so the 2x2 maxpool is three partition-aligned tensor_max ops.
  * fc1 weights are host-permuted to the pooled-feature order, so fc1 is
    24 accumulating K=120 matmuls per 125-neuron M-tile.
  * conv bias is folded into the fc1 bias on the host (maxpool commutes
    with the per-channel constant).
  * fc2 runs batch-major (stationary operand = activations) so softmax
    reduces along the free dim; fc2 bias via a K=1 ones matmul.
  * Matmuls use float32r (1 cycle/row at N>=256 vs 4 for fp32).
"""

import numpy as np

N, H, W = 8192, 28, 28
COUT, KS = 20, 5
NCORES = 8
NPC = N // NCORES  # images per core
CONV_W_OUT = 24
PH = 12            # pooled rows
FC1_IN, FC1_OUT, FC2_OUT = 2880, 500, 10
MT, MTS = 4, 125   # fc1 M tiles
KB, KBS = 24, 120  # a1 feature blocks (one per (pooled row, column half))

_cache = {}


def _build(npc, nb):
    from contextlib import ExitStack

    import concourse.tile as tile
    from concourse import bacc, mybir

    f32 = mybir.dt.float32
    f32r = mybir.dt.float32r
    nbt = npc // nb

    nc = bacc.Bacc(
        "TRN2",
        target_bir_lowering=False,
        debug=False,
        enable_asserts=False,
        num_devices=NCORES,
    )

    # host-im2col'd input: xg[jb, p, r, b] = x[(r + p//16)*28 + 12*jb + p%16, b]
    xg_d = nc.dram_tensor(
        "xg", [2, 80, CONV_W_OUT, npc], f32r, kind="ExternalInput"
    ).ap()
    t_d = nc.dram_tensor("tmat", [80, 240], f32r, kind="ExternalInput").ap()
    w1_d = nc.dram_tensor(
        "w1", [KB // 4, KBS, 4 * FC1_OUT], f32r, kind="ExternalInput"
    ).ap()
    b1_d = nc.dram_tensor("b1", [MTS, MT], f32, kind="ExternalInput").ap()
    w2_d = nc.dram_tensor("w2", [MTS, MT * FC2_OUT], f32r, kind="ExternalInput").ap()
    b2_d = nc.dram_tensor("b2", [FC2_OUT, 1], f32, kind="ExternalInput").ap()
    o_d = nc.dram_tensor("out", [npc, FC2_OUT], f32, kind="ExternalOutput").ap()

    with tile.TileContext(nc) as tc, ExitStack() as ctx:
        const = ctx.enter_context(tc.tile_pool(name="const", bufs=1))
        w1pool = ctx.enter_context(tc.tile_pool(name="w1", bufs=6))
        gpool = ctx.enter_context(tc.tile_pool(name="gather", bufs=8))
        a1pool = ctx.enter_context(tc.tile_pool(name="a1", bufs=8))
        tmppool = ctx.enter_context(tc.tile_pool(name="ptmp", bufs=4))
        a2pool = ctx.enter_context(tc.tile_pool(name="a2", bufs=2 * MT))
        smpool = ctx.enter_context(tc.tile_pool(name="softmax", bufs=4))
        cpsum = ctx.enter_context(tc.tile_pool(name="cpsum", bufs=4, space="PSUM"))
        fpsum = ctx.enter_context(tc.tile_pool(name="fpsum", bufs=4, space="PSUM"))

        from concourse.masks import make_identity

        t240 = const.tile([80, 240], f32r)
        nc.sync.dma_start(t240[:], t_d[:])
        # fc1 weights: 6 grouped DMAs of 4 blocks each, host-packed so every
        # group is one fully-contiguous [120, 2000] transfer.  Issued from
        # the compute-free gpsimd engine: issuing these from sync or scalar
        # blocks the gathers / pool-eviction copies behind the weight stream
        # and stalls the whole conv pipeline.
        WG = 4
        w1g = []
        for gidx in range(KB // WG):
            wt = w1pool.tile([KBS, WG * FC1_OUT], f32r, tag="w1",
                             name=f"w1g{gidx}")
            nc.gpsimd.dma_start(wt[:], w1_d[gidx])
            w1g.append(wt)
        b1t = const.tile([MTS, MT], f32)
        nc.scalar.dma_start(b1t[:], b1_d[:])
        w2t = const.tile([MTS, MT * FC2_OUT], f32r)
        nc.scalar.dma_start(w2t[:], w2_d[:])
        b2t = const.tile([FC2_OUT, 1], f32)
        nc.scalar.dma_start(b2t[:], b2_d[:])
        ident = const.tile([FC2_OUT, FC2_OUT], f32)
        make_identity(nc, ident[:])

        def w1_slice(j, mt):
            return w1g[j // WG][
                :, (j % WG) * FC1_OUT + mt * MTS : (j % WG) * FC1_OUT + (mt + 1) * MTS
            ]

        for bt in range(nbt):
            b0 = bt * nb
            a1 = [None] * KB
            # fc1 accumulators for all 4 M-tiles ride along with the conv
            # loop, skewed by 2 blocks: 4 dependency-free fc1 matmuls per
            # quad keep the PE gap-free so HAM stays at full clock.
            fp = [
                fpsum.tile([MTS, nb], f32, tag="fps", name=f"fp{bt}_{mt}")
                for mt in range(MT)
            ]
            SKEW = 4
            for kb in range(KB + SKEW):
                if kb >= SKEW:
                    j = kb - SKEW
                    for mt in range(MT):
                        nc.tensor.matmul(
                            fp[mt][:],
                            w1_slice(j, mt),
                            a1[j][:],
                            start=(j == 0),
                            stop=(j == KB - 1),
                        )
                if kb >= KB:
                    continue
                ip, jb = kb // 2, kb % 2
                g = []
                for dr in range(2):
                    gt = gpool.tile([80, nb], f32r, tag="g")
                    r = 2 * ip + dr
                    nc.sync.dma_start(gt[:], xg_d[jb, :, r, b0 : b0 + nb])
                    g.append(gt)
                ps = [
                    cpsum.tile([KBS, nb], f32, tag="cps", name=f"cps{i}")
                    for i in range(4)
                ]
                for dr in range(2):
                    for eo in range(2):
                        nc.tensor.matmul(
                            ps[2 * dr + eo][:],
                            t240[:, 120 * eo : 120 * (eo + 1)],
                            g[dr][:],
                            start=True,
                            stop=True,
                        )
                s0 = tmppool.tile([KBS, nb], f32, tag="s")
                nc.scalar.copy(s0[:], ps[0][:])
                m0 = tmppool.tile([KBS, nb], f32, tag="m")
                nc.vector.tensor_max(m0[:], s0[:], ps[1][:])
                s1 = tmppool.tile([KBS, nb], f32, tag="s")
                nc.scalar.copy(s1[:], ps[2][:])
                m1 = tmppool.tile([KBS, nb], f32, tag="m")
                nc.vector.tensor_max(m1[:], s1[:], ps[3][:])
                ab = a1pool.tile([KBS, nb], f32r, tag="a1")
                nc.vector.tensor_max(ab[:], m0[:], m1[:])
                a1[kb] = ab

            a2t = [None] * MT
            for mt in range(MT):
                a2 = a2pool.tile([MTS, nb], f32r, tag="a2")
                nc.scalar.activation(
                    a2[:],
                    fp[mt][:],
                    mybir.ActivationFunctionType.Relu,
                    bias=b1t[:, mt : mt + 1],
                )
                a2t[mt] = a2

            # fc2 feature-major: weights stationary, batch streams; softmax
            # needs batch on partitions, so PE-transpose 128-wide slices.
            p2f = fpsum.tile([FC2_OUT, nb], f32, tag="fps", name=f"p2f_{bt}")
            for mt in range(MT):
                nc.tensor.matmul(
                    p2f[:],
                    w2t[:, mt * FC2_OUT : (mt + 1) * FC2_OUT],
                    a2t[mt][:],
                    start=(mt == 0),
                    stop=(mt == MT - 1),
                )
            s2 = smpool.tile([FC2_OUT, nb], f32, tag="s2")
            nc.scalar.activation(
                s2[:], p2f[:], mybir.ActivationFunctionType.Identity,
                bias=b2t[:, 0:1],
            )
            sub = min(128, nb)
            for s in range(nb // sub):
                tp = fpsum.tile([sub, FC2_OUT], f32, tag="fps",
                                name=f"tp_{bt}_{s}")
                nc.tensor.transpose(
                    tp[:], s2[:, s * sub : (s + 1) * sub], ident[:]
                )
                e = smpool.tile([sub, FC2_OUT], f32, tag="e")
                ssum = smpool.tile([sub, 1], f32, tag="ss")
                nc.scalar.activation(
                    e[:], tp[:], mybir.ActivationFunctionType.Exp,
                    accum_out=ssum[:],
                )
                rinv = smpool.tile([sub, 1], f32, tag="ri")
                nc.vector.reciprocal(rinv[:], ssum[:])
                ot = smpool.tile([sub, FC2_OUT], f32, tag="ot")
                nc.vector.tensor_scalar_mul(ot[:], e[:], rinv[:])
                nc.sync.dma_start(o_d[b0 + s * sub : b0 + (s + 1) * sub, :], ot[:])

    nc.compile()
    return nc


def _prep_weights(conv_w, conv_b, fc1_w, fc1_b, fc2_w, fc2_b):
    conv_w = np.asarray(conv_w, np.float32).reshape(COUT, KS, KS)
    conv_b = np.asarray(conv_b, np.float32)
    fc1_w = np.asarray(fc1_w, np.float32)
    fc1_b = np.asarray(fc1_b, np.float32)
    fc2_w = np.asarray(fc2_w, np.float32)
    fc2_b = np.asarray(fc2_b, np.float32)

    # Toeplitz conv matrix [80, 240]: row = di*16 + jjp (input row offset,
    # input col within 16-wide block); col m = eo*120 + c*6 + q for output
    # col jj = 2q + eo within the 12-wide block.
    T = np.zeros((80, 240), np.float32)
    for m in range(240):
        eo, c, q = m // 120, (m % 120) // 6, m % 6
        jj = 2 * q + eo
        for di in range(KS):
            for dj in range(KS):
                T[di * 16 + jj + dj, m] = conv_w[c, di, dj]

    # fc1 weights permuted to our pooled-feature order:
    # block kb = ip*2 + jb, within-block m = c*6 + q
    # -> original flat feature c*144 + ip*12 + jb*6 + q
    kbv = np.arange(KB)
    ipv, jbv = kbv // 2, kbv % 2
    ml = np.arange(KBS)
    cv, qv = ml // 6, ml % 6
    fidx = cv[None, :] * 144 + ipv[:, None] * 12 + jbv[:, None] * 6 + qv[None, :]
    w1 = fc1_w.T[fidx.reshape(-1)].reshape(KB, KBS, FC1_OUT)
    # pack into 6 contiguous groups of 4 blocks: [6, 120, 4*500]
    w1 = np.ascontiguousarray(
        w1.reshape(KB // 4, 4, KBS, FC1_OUT).transpose(0, 2, 1, 3)
    ).reshape(KB // 4, KBS, 4 * FC1_OUT)

    # conv bias folded into fc1 bias (pool-max commutes with per-channel const)
    cb_vec = np.repeat(conv_b, 144)
    b1p = fc1_b + fc1_w @ cb_vec
    b1 = np.ascontiguousarray(b1p.reshape(MT, MTS).T)

    w2 = np.ascontiguousarray(
        fc2_w.T.reshape(MT, MTS, FC2_OUT).transpose(1, 0, 2)
    ).reshape(MTS, MT * FC2_OUT)
    b2 = np.ascontiguousarray(fc2_b.reshape(FC2_OUT, 1))
    return T, w1, b1, w2, b2


# im2col pixel indices: idx[jb, di*16+jjp, r] = (r+di)*28 + 12*jb + jjp
_IDX = np.zeros((2, 80, CONV_W_OUT), np.int64)
for _jb in range(2):
    for _di in range(KS):
        for _jjp in range(16):
            for _r in range(CONV_W_OUT):
                _IDX[_jb, _di * 16 + _jjp, _r] = (_r + _di) * W + 12 * _jb + _jjp


def _prep_x(x_core):
    """x_core [784, npc] pixel-major -> xg [2, 80, 24, npc]."""
    return np.ascontiguousarray(x_core[_IDX.reshape(-1)].reshape(
        2, 80, CONV_W_OUT, x_core.shape[1]))


def _run(inputs, npc=NPC, nb=512, trace=False):
    from concourse import bass_utils

    key = (npc, nb)
    if key not in _cache:
        _cache[key] = _build(npc, nb)
    nc = _cache[key]

    T, w1, b1, w2, b2 = _prep_weights(
        inputs["conv_w"], inputs["conv_b"], inputs["fc1_w"],
        inputs["fc1_b"], inputs["fc2_w"], inputs["fc2_b"],
    )
    x = np.asarray(inputs["x"], np.float32).reshape(-1, H * W)
    n_total = x.shape[0]
    assert n_total == NCORES * npc
    xs = x.reshape(NCORES, npc, H * W).transpose(0, 2, 1)

    in_maps = [
        {"xg": _prep_x(xs[i]), "tmat": T, "w1": w1, "b1": b1, "w2": w2,
         "b2": b2}
        for i in range(NCORES)
    ]
    res = bass_utils.run_bass_kernel_spmd(
        nc, in_maps, core_ids=list(range(NCORES)), trace=trace
    )
    out = np.concatenate([res.results[i]["out"] for i in range(NCORES)], axis=0)
    return out, res


def kernel(**inputs):
    out, _ = _run(inputs)
    return out



# revision 3
# speedup vs baseline: 1.3269x; 1.3269x over previous
"""Trainium2 Bass kernel for nn_CNNVectorForm (LeNet-style CNN, batch 8192).

Pipeline per core (data-parallel over batch, 1024 images/core):
  conv 5x5 VALID (1->20ch, 28->24)  -> 2x2 maxpool -> fc1(2880->500) + relu
  -> fc2(500->10) + softmax

Device formulation (v2, bf16 datapath):
  * All activations feature-major [features, batch]; batch rides the free
    dim (512 per tile).  Weights and activations are bf16 (1 cycle/row on
    the PE, half the HBM traffic of fp32); PSUM accumulation is fp32.
  * Conv as a Toeplitz matmul over merged 6-row gathers: per pooled row
    ip and column half jb one [96, nb] gather (6 input rows x 16 cols)
    feeds 4 matmuls (output row parity dr x column parity eo) with
    zero-padded stationaries T4[dr,eo] [96, 120].  Merging the rows cuts
    gather traffic 40% and halves the DMA instruction count vs per-row
    gathers.
  * 2x2 maxpool: scalar engine evacuates ps0/ps2 (PSUM->SBUF), vector
    does the two width maxes, gpsimd (idle otherwise) does the final
    height max, writing a1 in bf16.  Three engines run in parallel and
    each stays under the PE's 8-matmul-per-kb budget.
  * fc1 weights host-permuted to pooled-feature order; fc1 rides the conv
    loop skewed by SKEW blocks (4 dependency-free matmuls per quad).
  * conv bias folded into the fc1 bias on the host.
  * fc2 feature-major (4 accumulating K=125 matmuls), bias via vector
    tensor_scalar_add, PE-transpose 128-wide slices for the softmax,
    results staged in one [128, 40] tile -> single output DMA per tile.
"""

import numpy as np
import ml_dtypes

N, H, W = 8192, 28, 28
COUT, KS = 20, 5
NCORES = 8
NPC = N // NCORES  # images per core
PH = 12            # pooled rows
FC1_IN, FC1_OUT, FC2_OUT = 2880, 500, 10
MT, MTS = 4, 125   # fc1 M tiles
KB, KBS = 24, 120  # a1 feature blocks (one per (pooled row, column half))
SKEW = 4           # fc1 trails conv by SKEW blocks
GROWS = 96         # merged gather rows: 6 input rows x 16 cols

BF16 = ml_dtypes.bfloat16

_cache = {}


def _build(npc, nb):
    from contextlib import ExitStack

    import concourse.tile as tile
    from concourse import bacc, mybir

    f32 = mybir.dt.float32
    bf16 = mybir.dt.bfloat16
    nbt = npc // nb

    nc = bacc.Bacc(
        "TRN2",
        target_bir_lowering=False,
        debug=False,
        enable_asserts=False,
        num_devices=NCORES,
    )

    # host-im2col'd input: xg[jb, p, ip, b] = x[(2*ip + p//16)*28 + 12*jb + p%16, b]
    xg_d = nc.dram_tensor(
        "xg", [2, GROWS, PH, npc], bf16, kind="ExternalInput"
    ).ap()
    t4_d = nc.dram_tensor("t4", [GROWS, 4 * KBS], bf16, kind="ExternalInput").ap()
    w1_d = nc.dram_tensor(
        "w1", [KB // 4, KBS, 4 * FC1_OUT], bf16, kind="ExternalInput"
    ).ap()
    b1_d = nc.dram_tensor("b1", [MTS, MT], f32, kind="ExternalInput").ap()
    w2_d = nc.dram_tensor("w2", [MTS, MT * FC2_OUT], bf16, kind="ExternalInput").ap()
    b2_d = nc.dram_tensor("b2", [FC2_OUT, 1], f32, kind="ExternalInput").ap()
    o_d = nc.dram_tensor("out", [npc, FC2_OUT], f32, kind="ExternalOutput").ap()

    with tile.TileContext(nc) as tc, ExitStack() as ctx:
        const = ctx.enter_context(tc.tile_pool(name="const", bufs=1))
        w1pool = ctx.enter_context(tc.tile_pool(name="w1", bufs=6))
        gpool = ctx.enter_context(tc.tile_pool(name="gather", bufs=8))
        a1pool = ctx.enter_context(tc.tile_pool(name="a1", bufs=SKEW + 4))
        tmppool = ctx.enter_context(tc.tile_pool(name="ptmp", bufs=6))
        a2pool = ctx.enter_context(tc.tile_pool(name="a2", bufs=2 * MT))
        smpool = ctx.enter_context(tc.tile_pool(name="softmax", bufs=4))
        cpsum = ctx.enter_context(tc.tile_pool(name="cpsum", bufs=4, space="PSUM"))
        fpsum = ctx.enter_context(tc.tile_pool(name="fpsum", bufs=4, space="PSUM"))

        from concourse.masks import make_identity

        # conv stationaries first on sync so the first matmul can start ASAP
        t4 = const.tile([GROWS, 4 * KBS], bf16)
        nc.sync.dma_start(t4[:], t4_d[:])
        # fc1 weights: 6 grouped DMAs, host-packed so every group is one
        # fully-contiguous [120, 2000] transfer, streamed from gpsimd's
        # queue so they don't block the gather stream on sync.
        WG = 4
        w1g = []
        for gidx in range(KB // WG):
            wt = w1pool.tile([KBS, WG * FC1_OUT], bf16, tag="w1",
                             name=f"w1g{gidx}")
            nc.gpsimd.dma_start(wt[:], w1_d[gidx])
            w1g.append(wt)
        b1t = const.tile([MTS, MT], f32)
        nc.scalar.dma_start(b1t[:], b1_d[:])
        w2t = const.tile([MTS, MT * FC2_OUT], bf16)
        nc.scalar.dma_start(w2t[:], w2_d[:])
        b2t = const.tile([FC2_OUT, 1], f32)
        nc.scalar.dma_start(b2t[:], b2_d[:])
        ident = const.tile([FC2_OUT, FC2_OUT], f32)
        make_identity(nc, ident[:])

        def w1_slice(j, mt):
            return w1g[j // WG][
                :, (j % WG) * FC1_OUT + mt * MTS : (j % WG) * FC1_OUT + (mt + 1) * MTS
            ]

        for bt in range(nbt):
            b0 = bt * nb
            a1 = [None] * KB
            # fc1 accumulators for all 4 M-tiles ride along with the conv
            # loop, skewed by SKEW blocks: 4 dependency-free fc1 matmuls
            # per quad keep the PE gap-free so HAM stays at full clock.
            fp = [
                fpsum.tile([MTS, nb], f32, tag="fps", name=f"fp{bt}_{mt}")
                for mt in range(MT)
            ]
            for kb in range(KB + SKEW):
                if kb >= SKEW:
                    j = kb - SKEW
                    for mt in range(MT):
                        nc.tensor.matmul(
                            fp[mt][:],
                            w1_slice(j, mt),
                            a1[j][:],
                            start=(j == 0),
                            stop=(j == KB - 1),
                        )
                if kb >= KB:
                    continue
                ip, jb = kb // 2, kb % 2
                g = gpool.tile([GROWS, nb], bf16, tag="g")
                nc.sync.dma_start(g[:], xg_d[jb, :, ip, b0 : b0 + nb])
                ps = [
                    cpsum.tile([KBS, nb], f32, tag="cps", name=f"cps{i}")
                    for i in range(4)
                ]
                for dr in range(2):
                    for eo in range(2):
                        nc.tensor.matmul(
                            ps[2 * dr + eo][:],
                            t4[:, (2 * dr + eo) * KBS : (2 * dr + eo + 1) * KBS],
                            g[:],
                            start=True,
                            stop=True,
                        )
                # 2x2 maxpool: scalar evacuates the even-parity PSUM banks,
                # vector does the width maxes (PSUM-limited 1x) writing
                # bf16, so the final height max runs in the DVE's 2x_1P
                # packed mode at half cost.
                s0 = tmppool.tile([KBS, nb], f32, tag="s")
                nc.scalar.copy(s0[:], ps[0][:])
                m0 = tmppool.tile([KBS, nb], bf16, tag="m")
                nc.vector.tensor_max(m0[:], s0[:], ps[1][:])
                s1 = tmppool.tile([KBS, nb], f32, tag="s")
                nc.scalar.copy(s1[:], ps[2][:])
                m1 = tmppool.tile([KBS, nb], bf16, tag="m")
                nc.vector.tensor_max(m1[:], s1[:], ps[3][:])
                ab = a1pool.tile([KBS, nb], bf16, tag="a1")
                nc.vector.tensor_max(ab[:], m0[:], m1[:])
                a1[kb] = ab

            a2t = [None] * MT
            for mt in range(MT):
                a2 = a2pool.tile([MTS, nb], bf16, tag="a2")
                nc.scalar.activation(
                    a2[:],
                    fp[mt][:],
                    mybir.ActivationFunctionType.Relu,
                    bias=b1t[:, mt : mt + 1],
                )
                a2t[mt] = a2

            # fc2 feature-major: weights stationary, batch streams; softmax
            # needs batch on partitions, so PE-transpose 128-wide slices.
            p2f = fpsum.tile([FC2_OUT, nb], f32, tag="fps", name=f"p2f_{bt}")
            for mt in range(MT):
                nc.tensor.matmul(
                    p2f[:],
                    w2t[:, mt * FC2_OUT : (mt + 1) * FC2_OUT],
                    a2t[mt][:],
                    start=(mt == 0),
                    stop=(mt == MT - 1),
                )
            s2 = smpool.tile([FC2_OUT, nb], f32, tag="s2")
            nc.vector.tensor_scalar_add(s2[:], p2f[:], b2t[:, 0:1])
            sub = min(128, nb)
            nsub = nb // sub
            stage = smpool.tile([sub, nsub * FC2_OUT], f32, tag="ot")
            for s in range(nsub):
                tp = fpsum.tile([sub, FC2_OUT], f32, tag="fps",
                                name=f"tp_{bt}_{s}")
                nc.tensor.transpose(
                    tp[:], s2[:, s * sub : (s + 1) * sub], ident[:]
                )
                e = smpool.tile([sub, FC2_OUT], f32, tag="e")
                ssum = smpool.tile([sub, 1], f32, tag="ss")
                nc.scalar.activation(
                    e[:], tp[:], mybir.ActivationFunctionType.Exp,
                    accum_out=ssum[:],
                )
                rinv = smpool.tile([sub, 1], f32, tag="ri")
                nc.vector.reciprocal(rinv[:], ssum[:])
                nc.vector.tensor_scalar_mul(
                    stage[:, s * FC2_OUT : (s + 1) * FC2_OUT], e[:], rinv[:]
                )
            dst = o_d[b0 : b0 + nb, :].rearrange(
                "(s p) c -> p s c", s=nsub, p=sub
            )
            src = stage[:].rearrange("p (s c) -> p s c", s=nsub, c=FC2_OUT)
            nc.sync.dma_start(dst, src)

    nc.compile()
    return nc


def _prep_weights(conv_w, conv_b, fc1_w, fc1_b, fc2_w, fc2_b):
    conv_w = np.asarray(conv_w, np.float32).reshape(COUT, KS, KS)
    conv_b = np.asarray(conv_b, np.float32)
    fc1_w = np.asarray(fc1_w, np.float32)
    fc1_b = np.asarray(fc1_b, np.float32)
    fc2_w = np.asarray(fc2_w, np.float32)
    fc2_b = np.asarray(fc2_b, np.float32)

    # Toeplitz conv matrices [96, 4*120]: four stationaries (dr, eo) over a
    # merged 6-row x 16-col gather; col m = (2*dr+eo)*120 + c*6 + q maps to
    # conv output (row 2*ip+dr, col 12*jb + 2*q+eo, channel c).
    T4 = np.zeros((GROWS, 4 * KBS), np.float32)
    for dr in range(2):
        for eo in range(2):
            for c in range(COUT):
                for q in range(6):
                    m = (2 * dr + eo) * KBS + c * 6 + q
                    for di in range(KS):
                        for dj in range(KS):
                            T4[(di + dr) * 16 + 2 * q + eo + dj, m] = conv_w[c, di, dj]

    # fc1 weights permuted to our pooled-feature order:
    # block kb = ip*2 + jb, within-block m = c*6 + q
    # -> original flat feature c*144 + ip*12 + jb*6 + q
    kbv = np.arange(KB)
    ipv, jbv = kbv // 2, kbv % 2
    ml = np.arange(KBS)
    cv, qv = ml // 6, ml % 6
    fidx = cv[None, :] * 144 + ipv[:, None] * 12 + jbv[:, None] * 6 + qv[None, :]
    w1 = fc1_w.T[fidx.reshape(-1)].reshape(KB, KBS, FC1_OUT)
    # pack into 6 contiguous groups of 4 blocks: [6, 120, 4*500]
    w1 = np.ascontiguousarray(
        w1.reshape(KB // 4, 4, KBS, FC1_OUT).transpose(0, 2, 1, 3)
    ).reshape(KB // 4, KBS, 4 * FC1_OUT)

    # conv bias folded into fc1 bias (pool-max commutes with per-channel const)
    cb_vec = np.repeat(conv_b, 144)
    b1p = fc1_b + fc1_w @ cb_vec
    b1 = np.ascontiguousarray(b1p.reshape(MT, MTS).T)

    w2 = np.ascontiguousarray(
        fc2_w.T.reshape(MT, MTS, FC2_OUT).transpose(1, 0, 2)
    ).reshape(MTS, MT * FC2_OUT)
    b2 = np.ascontiguousarray(fc2_b.reshape(FC2_OUT, 1))
    return (T4.astype(BF16), w1.astype(BF16), b1,
            w2.astype(BF16), b2)


# im2col pixel indices: idx[jb, p, ip] = (2*ip + p//16)*28 + 12*jb + p%16
_IDX = np.zeros((2, GROWS, PH), np.int64)
for _jb in range(2):
    for _di in range(6):
        for _jjp in range(16):
            for _ip in range(PH):
                _IDX[_jb, _di * 16 + _jjp, _ip] = (2 * _ip + _di) * W + 12 * _jb + _jjp


def _prep_x(x_core):
    """x_core [784, npc] pixel-major -> xg [2, 96, 12, npc] bf16."""
    return np.ascontiguousarray(x_core[_IDX.reshape(-1)].reshape(
        2, GROWS, PH, x_core.shape[1]).astype(BF16))


def _feeds(inputs, npc):
    """Per-core feed dicts for the full batch (list of NCORES dicts)."""
    T4, w1, b1, w2, b2 = _prep_weights(
        inputs["conv_w"], inputs["conv_b"], inputs["fc1_w"],
        inputs["fc1_b"], inputs["fc2_w"], inputs["fc2_b"],
    )
    x = np.asarray(inputs["x"], np.float32).reshape(-1, H * W)
    n_total = x.shape[0]
    assert n_total == NCORES * npc
    xs = x.reshape(NCORES, npc, H * W).transpose(0, 2, 1)
    return [
        {"xg": _prep_x(xs[i]), "t4": T4, "w1": w1, "b1": b1, "w2": w2,
         "b2": b2}
        for i in range(NCORES)
    ]


def _run(inputs, npc=NPC, nb=512, trace=False):
    from concourse import bass_utils

    key = (npc, nb)
    if key not in _cache:
        _cache[key] = _build(npc, nb)
    nc = _cache[key]

    in_maps = _feeds(inputs, npc)
    res = bass_utils.run_bass_kernel_spmd(
        nc, in_maps, core_ids=list(range(NCORES)), trace=trace
    )
    out = np.concatenate([res.results[i]["out"] for i in range(NCORES)], axis=0)
    return out, res


def kernel(**inputs):
    out, _ = _run(inputs)
    return out
